# revision 1
# baseline (speedup 1.0000x reference)
"""Causal self-attention for Trainium2, 8 NeuronCores.

Sharding: tensor-parallel over heads (4 heads/core) x data-parallel over
batch (2). Core i handles batch i//4, heads 4*(i%4)..4*(i%4)+3. Each core
computes its heads' attention output and a partial output projection
(W_proj rows for its heads); the host sums the 4 partials per batch and
adds b_proj.

Device layout choices:
  - Q^T, K^T computed feature-major [dim, t] directly (lhsT = W chunk,
    rhs = x^T chunk), so attention scores come out as S^T [k, q] with k
    on partitions -- which is exactly the layout the P@V matmul needs
    as its rhs. No on-chip transposes of the O(T^2) object.
  - V computed in natural [t, dim] layout (lhsT = x^T chunk, rhs = W_v),
    which is the lhsT layout the P@V matmul needs. A ones-column is
    appended to V so the softmax denominators fall out of the same
    matmul (row 64*... of the PSUM output).
  - exp() without max subtraction: scores are q.k/8 with q,k ~ N(0,1),
    bounded well inside fp32 exp range; softmax is shift-invariant so
    the result is mathematically identical to the reference.
  - all matmuls run as float32r (replicated fp32) with free dim >= 256,
    which streams at 1 column/cycle like bf16.

The causal mask is handled by skipping fully-masked k-chunks and
multiplying exp(S) by one of 4 precomputed 0/1 indicator tiles on the
diagonal-straddling chunks. If the runtime mask is not the lower-tri
causal mask, a general fallback multiplies by the actual mask (DMA'd
transposed) instead; an all-ones mask drops masking entirely.
"""

import numpy as np

B, T, C, H = 2, 2048, 1024, 16
D = C // H            # 64 head dim
NCORES = 8
NBG = 2               # batch shards
NHG = 4               # head-group shards
HL = H // NHG         # 4 heads per core
DL = HL * D           # 256 local feature dims
NDQ = DL // 128       # 2 partition chunks of local dims
NTB = T // 512        # 4 t-chunks of 512
NKC = T // 128        # 16 key chunks of 128
NQC = T // 512        # 4 query chunks of 512
NTT = T // 128        # 16 t-tiles of 128 (proj / V)

_CACHE = {}


def _build(mode, debug_dump=False):
    """Build + compile the per-core Bass program. mode: causal|full|general."""
    import concourse.bass as bass
    import concourse.bacc as bacc
    import concourse.tile as tile
    import concourse.mybir as mybir

    f32 = mybir.dt.float32
    bf16 = mybir.dt.bfloat16
    Exp = mybir.ActivationFunctionType.Exp
    Ident = mybir.ActivationFunctionType.Identity
    mult = mybir.AluOpType.mult
    add = mybir.AluOpType.add

    nc = bacc.Bacc(
        "TRN2", target_bir_lowering=False, debug=False, num_devices=NCORES
    )

    xT = nc.dram_tensor("xT", [C, T], bf16, kind="ExternalInput").ap()
    Wl = nc.dram_tensor("Wl", [C, 3 * DL], bf16, kind="ExternalInput").ap()
    bqk = nc.dram_tensor("bqk", [128, 2 * NDQ], f32, kind="ExternalInput").ap()
    bv = nc.dram_tensor("bv", [1, DL], f32, kind="ExternalInput").ap()
    Wp = nc.dram_tensor("Wp", [DL, C], bf16, kind="ExternalInput").ap()
    maskT = None
    if mode == "general":
        maskT = nc.dram_tensor("maskT", [T, T], bf16, kind="ExternalInput").ap()
    yp = nc.dram_tensor("yp", [T, C], f32, kind="ExternalOutput").ap()
    dbg = {}
    if debug_dump:
        for nm, shp, dt in [
            ("qt_d", [128, NDQ, T], bf16), ("kt_d", [128, NDQ, T], bf16),
            ("v1_d", [128, NKC, HL, D + 1], bf16), ("ot_d", [128, NDQ, T], bf16),
            ("st_d", [128, 512], f32), ("p_d", [128, 512], bf16),
            ("o_d", [65, 512], f32),
        ]:
            dbg[nm] = nc.dram_tensor(nm, shp, dt, kind="ExternalOutput").ap()

    with tile.TileContext(nc) as tc:
        with (
            tc.tile_pool(name="singles", bufs=1) as singles,
            tc.tile_pool(name="xin", bufs=2) as xin,
            tc.tile_pool(name="ptiles", bufs=6) as ptiles,
            tc.tile_pool(name="small", bufs=4) as small,
            tc.tile_pool(name="outp", bufs=3) as outp,
            tc.tile_pool(name="psum", bufs=7, space="PSUM") as psum,
        ):
            def ps512(name):
                return psum.tile(
                    [128, 512], f32, name="ps512", tag="ps512", bufs=4
                )

            # ---- resident inputs ----
            # W and x loads split per kc-chunk so the first matmuls can
            # start as soon as their chunk lands.
            W_sb = singles.tile([128, 8, 3 * DL], bf16)
            Wl_r = Wl.rearrange("(kc p) n -> p kc n", p=128)
            x0_sb = xin.tile([128, 8, 512], bf16, tag="x_sb", name="x_sb")
            x0r = xT.rearrange("(kc p) t -> p kc t", p=128)[:, :, 0:512]
            for kc in range(8):
                nc.sync.dma_start(out=W_sb[:, kc, :], in_=Wl_r[:, kc, :])
                nc.sync.dma_start(out=x0_sb[:, kc, :], in_=x0r[:, kc, :])
            bqk_sb = singles.tile([128, 2 * NDQ], f32)
            nc.sync.dma_start(out=bqk_sb, in_=bqk)
            bv_row = singles.tile([1, DL], f32)
            nc.sync.dma_start(out=bv_row, in_=bv)
            bv_sb = singles.tile([128, DL], f32)
            nc.gpsimd.partition_broadcast(bv_sb, bv_row)

            ind = None
            if mode == "causal":
                ind = singles.tile([128, 4, 512], bf16)
                for j in range(4):
                    nc.vector.memset(ind[:, j, :], 1.0)
                    # keep (=1.0) iff f - p - 128*j >= 0, else 0.0
                    nc.gpsimd.affine_select(
                        out=ind[:, j, :],
                        in_=ind[:, j, :],
                        compare_op=mybir.AluOpType.is_ge,
                        fill=0.0,
                        base=-128 * j,
                        pattern=[[1, 512]],
                        channel_multiplier=-1,
                    )

            # ---- resident intermediates ----
            QT = singles.tile([128, NDQ, T], bf16)   # [dim%128, dimchunk, t]
            KT = singles.tile([128, NDQ, T], bf16)
            V1 = singles.tile([128, NKC, HL, D + 1], bf16)  # [t%128, kc, h, d+1]
            nc.vector.memset(V1[:, :, :, D : D + 1], 1.0)
            OT = singles.tile([128, NDQ, T], bf16)
            stage_sb = singles.tile([65, NQC, HL, 512], f32)
            Wp_sb = singles.tile([128, NDQ, C], bf16)

            # ---- phase 1: QKV projections (as interleavable units) ----
            def p1_units(tb, x_sb):
                """Units for one 512-wide t-chunk of the QKV projection."""
                units = []
                if tb > 0:
                    def dma_u(tb=tb, x_sb=x_sb):
                        xr = xT.rearrange("(kc p) t -> p kc t", p=128)[
                            :, :, tb * 512 : (tb + 1) * 512
                        ]
                        for kc in range(8):
                            nc.sync.dma_start(
                                out=x_sb[:, kc, :], in_=xr[:, kc, :]
                            )
                    units.append(dma_u)
                for s in range(2):  # 0=Q, 1=K
                    for dq in range(NDQ):
                        def qk_u(tb=tb, s=s, dq=dq, x_sb=x_sb):
                            ps = ps512("qk")
                            col = s * DL + dq * 128
                            for kc in range(8):
                                nc.tensor.matmul(
                                    ps,
                                    lhsT=W_sb[:, kc, col : col + 128],
                                    rhs=x_sb[:, kc, :],
                                    start=(kc == 0),
                                    stop=(kc == 7),
                                )
                            dst = (QT if s == 0 else KT)[
                                :, dq, tb * 512 : (tb + 1) * 512
                            ]
                            nc.vector.tensor_scalar_add(
                                dst, ps,
                                bqk_sb[:, s * NDQ + dq : s * NDQ + dq + 1],
                            )
                        units.append(qk_u)
                for t4 in range(4):
                    def v_u(tb=tb, t4=t4, x_sb=x_sb):
                        tt = tb * 4 + t4
                        ps = ps512("v")
                        for kc in range(8):
                            nc.tensor.matmul(
                                ps[:, :DL],
                                lhsT=x_sb[:, kc, t4 * 128 : (t4 + 1) * 128],
                                rhs=W_sb[:, kc, 2 * DL : 3 * DL],
                                start=(kc == 0),
                                stop=(kc == 7),
                            )
                        nc.vector.tensor_tensor(
                            out=V1[:, tt, :, 0:D],
                            in0=ps[:, :DL].rearrange("p (h d) -> p h d", d=D),
                            in1=bv_sb.rearrange("p (h d) -> p h d", d=D),
                            op=add,
                        )
                    units.append(v_u)
                return units

            # ---- phase 2/3 units ----
            def proj_units(qc):
                units = []
                for t4 in range(4):
                    def u(qc=qc, t4=t4):
                        tt = qc * 4 + t4
                        y_sb = outp.tile([128, C], f32, name="y_sb")
                        for n in range(2):
                            pp = ps512("proj")
                            for dq in range(NDQ):
                                nc.tensor.matmul(
                                    pp,
                                    lhsT=OT[:, dq, tt * 128 : (tt + 1) * 128],
                                    rhs=Wp_sb[:, dq, n * 512 : (n + 1) * 512],
                                    start=(dq == 0),
                                    stop=(dq == NDQ - 1),
                                )
                            nc.vector.tensor_copy(
                                y_sb[:, n * 512 : (n + 1) * 512], pp
                            )
                        nc.sync.dma_start(
                            out=yp[tt * 128 : (tt + 1) * 128, :], in_=y_sb
                        )
                    units.append(u)
                return units

            def attn_units(qc, hp, nkc, m_sb):
                """One head-pair's attention over all k-chunks, softmax
                denominators via the ones-column of V1."""
                state = {}

                def emit_mm1(j):
                    stp = psum.tile(
                        [128, 2, 512], f32, name="ps1024", tag="ps1024", bufs=2
                    )
                    for hh in range(2):
                        off = 64 * hh
                        nc.tensor.matmul(
                            stp[:, hh, :],
                            lhsT=KT[off : off + 64, hp, j * 128 : (j + 1) * 128],
                            rhs=QT[
                                off : off + 64, hp, qc * 512 : (qc + 1) * 512
                            ],
                            start=True,
                            stop=True,
                        )
                    state.setdefault("st", {})[j] = stp

                def prologue():
                    state["ops"] = [ps512("o"), ps512("o")]
                    state["emitted"] = min(2, nkc)  # lookahead 1
                    for j in range(state["emitted"]):
                        emit_mm1(j)

                def consume(kc):
                    ops = state["ops"]
                    stp = state["st"].pop(kc)
                    p2 = ptiles.tile([128, 2, 512], bf16, tag="p")
                    if mode == "causal" and kc >= 4 * qc:
                        # exp only the columns the causal mask can reach;
                        # zero the fully-masked prefix, then apply the
                        # diagonal indicator to both heads at once.
                        j = kc - 4 * qc
                        if j > 0:
                            nc.vector.memset(p2[:, :, 0 : 128 * j], 0.0)
                        nc.scalar.activation(
                            p2[:, :, 128 * j :], stp[:, :, 128 * j :], Exp
                        )
                        base = ind[:, j, 128 * j :]
                        ind2 = bass.AP(
                            tensor=base.tensor,
                            offset=base.offset,
                            ap=[base.ap[0], [0, 2], base.ap[1]],
                        )
                        nc.vector.tensor_tensor(
                            out=p2[:, :, 128 * j :],
                            in0=p2[:, :, 128 * j :],
                            in1=ind2,
                            op=mult,
                        )
                    else:
                        nc.scalar.activation(p2, stp, Exp)
                        if mode == "general":
                            base = m_sb[:, kc, :]
                            msk2 = bass.AP(
                                tensor=base.tensor,
                                offset=base.offset,
                                ap=[base.ap[0], [0, 2], base.ap[1]],
                            )
                            nc.vector.tensor_tensor(
                                out=p2, in0=p2, in1=msk2, op=mult
                            )
                    for hh in range(2):
                        h = hp * 2 + hh
                        nc.tensor.matmul(
                            ops[hh][: D + 1, :],
                            lhsT=V1[:, kc, h, :],
                            rhs=p2[:, hh, :],
                            start=(kc == 0),
                            stop=(kc == nkc - 1),
                        )
                    if state["emitted"] < nkc:
                        emit_mm1(state["emitted"])
                        state["emitted"] += 1

                def stash():
                    # unnormalized output rows + denominators (the
                    # denominator row stays on partition 64 -- engines
                    # can't move data across partitions; DMA gathers it)
                    ops = state["ops"]
                    for hh in range(2):
                        h = hp * 2 + hh
                        off = 64 * hh
                        nc.vector.tensor_copy(
                            OT[off : off + 64, hp, qc * 512 : (qc + 1) * 512],
                            ops[hh][0:D, :],
                        )
                        nc.vector.tensor_copy(
                            stage_sb[64:65, qc, h, :], ops[hh][D : D + 1, :]
                        )

                units = [prologue]
                for kc in range(nkc):
                    units.append(lambda kc=kc: consume(kc))
                units.append(stash)
                return units

            # SBUF sources can't have partition-step 0; denominators bounce
            # through DRAM so the broadcast reads DRAM with a step-0 dim.
            rcp_dram = nc.dram_tensor(
                "rcp_scratch", [NQC, NDQ, NDQ, 512], bf16, kind="Internal"
            ).ap()

            def norm_hp(qc, hp):
                # per-(qc, head-pair) normalization; runs as soon as this
                # pair's denominators are stashed, fully off the PE path.
                sums2 = small.tile([NDQ, 512], f32, tag="sums2", bufs=2)
                nc.gpsimd.dma_start(
                    out=sums2, in_=stage_sb[64:65, qc, 2 * hp : 2 * hp + 2, :]
                )
                rcp2 = small.tile([NDQ, 512], f32, tag="rcp2", bufs=2)
                scr2 = small.tile([NDQ, 512], f32, tag="scr2", bufs=2)
                nc.vector.reciprocal_approx_accurate(
                    out=rcp2, in_=sums2, scratch=scr2
                )
                rcpb2 = small.tile([NDQ, 512], bf16, tag="rcpb2", bufs=2)
                nc.vector.tensor_copy(rcpb2, rcp2)
                nc.sync.dma_start(out=rcp_dram[qc, hp], in_=rcpb2)
                rb_hp = small.tile([128, 512], bf16, tag="rb", bufs=2)
                for hh in range(2):
                    src = rcp_dram[qc, hp, hh : hh + 1, :]
                    src = bass.AP(
                        tensor=src.tensor,
                        offset=src.offset,
                        ap=[[0, 64], src.ap[-1]],
                    )
                    nc.gpsimd.dma_start(
                        out=rb_hp[64 * hh : 64 * hh + 64, :], in_=src
                    )
                nc.vector.tensor_tensor(
                    out=OT[:, hp, qc * 512 : (qc + 1) * 512],
                    in0=OT[:, hp, qc * 512 : (qc + 1) * 512],
                    in1=rb_hp,
                    op=mult,
                )

            # ---- schedule: staircase interleave ----
            # attn(qc) needs phase-1 chunks tb <= qc only, so phase-1(tb+1)
            # and proj(qc-1) units are injected between attention units to
            # keep the PE FIFO fed while ACT paces the exp chain.
            for u in p1_units(0, x0_sb):
                u()
            nc.sync.dma_start(
                out=Wp_sb, in_=Wp.rearrange("(dq p) n -> p dq n", p=128)
            )
            for qc in range(NQC):
                nkc = 4 * qc + 4 if mode == "causal" else NKC
                m_sb = None
                if mode == "general":
                    m_sb = xin.tile([128, NKC, 512], bf16, tag="mask", bufs=1)
                    nc.sync.dma_start(
                        out=m_sb,
                        in_=maskT.rearrange("(kc p) q -> p kc q", p=128)[
                            :, :, qc * 512 : (qc + 1) * 512
                        ],
                    )
                inj_early = []
                if qc + 1 < NTB:
                    x_next = xin.tile(
                        [128, 8, 512], bf16, tag="x_sb", name="x_sb"
                    )
                    inj_early += p1_units(qc + 1, x_next)
                inj_late = proj_units(qc - 1) if qc >= 1 else []
                main = []
                for hp in range(NDQ):
                    units = attn_units(qc, hp, nkc, m_sb)
                    units.append(lambda qc=qc, hp=hp: norm_hp(qc, hp))
                    main += units
                half = (len(main) + 1) // 2
                for part, inj in ((main[:half], inj_early), (main[half:], inj_late)):
                    k, m, j = len(part), len(inj), 0
                    for i, u in enumerate(part):
                        u()
                        take = (i + 1) * m // k - i * m // k
                        for _ in range(take):
                            inj[j]()
                            j += 1
            for u in proj_units(NQC - 1):
                u()

            if debug_dump:
                nc.sync.dma_start(out=dbg["ot_d"], in_=OT)

    nc.compile()
    return nc


def _host_prep(x, prefix_causal_mask, W_attn, b_attn, W_proj):
    """Split full inputs into 8 per-core input maps; detect mask mode."""
    scale = 1.0 / np.sqrt(np.float32(D))
    mask = np.asarray(prefix_causal_mask)
    if mask.all():
        mode = "full"
    else:
        tri = np.tril(np.ones((T, T), dtype=bool))
        if all(np.array_equal(mask[b], tri) for b in range(B)):
            mode = "causal"
        else:
            mode = "general"

    import ml_dtypes

    bf16 = ml_dtypes.bfloat16
    x = np.asarray(x, dtype=np.float32)
    W_attn = np.asarray(W_attn, dtype=np.float32)
    b_attn = np.asarray(b_attn, dtype=np.float32)
    W_proj = np.asarray(W_proj, dtype=np.float32)

    in_maps = []
    for core in range(NCORES):
        b = core // NHG
        hg = core % NHG
        lo = hg * DL
        hi = lo + DL
        xT = np.ascontiguousarray(x[b].T)  # [C, T]
        Wq = W_attn[:, lo:hi] * scale
        Wk = W_attn[:, C + lo : C + hi]
        Wv = W_attn[:, 2 * C + lo : 2 * C + hi]
        Wl = np.ascontiguousarray(np.concatenate([Wq, Wk, Wv], axis=1))
        bq = b_attn[lo:hi] * scale
        bk = b_attn[C + lo : C + hi]
        # bias per partition for Q,K chunks: cols = [q0, q1, k0, k1]
        bqk = np.stack(
            [bq[0:128], bq[128:256], bk[0:128], bk[128:256]], axis=1
        ).astype(np.float32)
        bv = np.ascontiguousarray(
            b_attn[2 * C + lo : 2 * C + hi][None, :]
        ).astype(np.float32)
        Wp = np.ascontiguousarray(W_proj[lo:hi, :])
        im = {
            "xT": xT.astype(bf16),
            "Wl": Wl.astype(bf16),
            "bqk": np.ascontiguousarray(bqk),
            "bv": bv,
            "Wp": Wp.astype(bf16),
        }
        if mode == "general":
            im["maskT"] = np.ascontiguousarray(mask[b].T).astype(bf16)
        in_maps.append(im)
    return mode, in_maps


def _get_program(mode):
    if mode not in _CACHE:
        _CACHE[mode] = _build(mode)
    return _CACHE[mode]


def _run(inputs, trace=False):
    """Returns (full_output [B,T,C], BassKernelResults)."""
    from concourse import bass_utils

    mode, in_maps = _host_prep(
        inputs["x"],
        inputs["prefix_causal_mask"],
        inputs["W_attn"],
        inputs["b_attn"],
        inputs["W_proj"],
    )
    nc = _get_program(mode)
    res = bass_utils.run_bass_kernel_spmd(
        nc, in_maps, core_ids=list(range(NCORES)), trace=trace
    )
    b_proj = np.asarray(inputs["b_proj"], dtype=np.float32)
    y = np.zeros((B, T, C), dtype=np.float32)
    for core in range(NCORES):
        y[core // NHG] += res.results[core]["yp"]
    y += b_proj[None, None, :]
    return y, res


def kernel(**inputs):
    y, _ = _run(inputs, trace=False)
    return y



# revision 40
# speedup vs baseline: 1.1166x; 1.1166x over previous
"""Causal self-attention for Trainium2, 8 NeuronCores.

Sharding: tensor-parallel over heads (4 heads/core) x data-parallel over
batch (2). Core i handles batch i//4, heads 4*(i%4)..4*(i%4)+3. Each core
computes its heads' attention output and a partial output projection
(W_proj rows for its heads); the host sums the 4 partials per batch and
adds b_proj.

Device layout choices:
  - Q^T, K^T computed feature-major [dim, t] directly (lhsT = W chunk,
    rhs = x^T chunk), so attention scores come out as S^T [k, q] with k
    on partitions -- which is exactly the layout the P@V matmul needs
    as its rhs. No on-chip transposes of the O(T^2) object.
  - V computed in natural [t, dim] layout (lhsT = x^T chunk, rhs = W_v),
    which is the lhsT layout the P@V matmul needs. A ones-column is
    appended to V so the softmax denominators fall out of the same
    matmul (row 64*... of the PSUM output).
  - exp() without max subtraction: scores are q.k/8 with q,k ~ N(0,1),
    bounded well inside fp32 exp range; softmax is shift-invariant so
    the result is mathematically identical to the reference.
  - diagonal-straddling k-chunks only compute/exp/accumulate score
    columns >= 128*j (the reachable ones); the 128-wide straddle block
    gets a precomputed 0/1 indicator multiply; fully-masked prefixes are
    never touched (no memsets).
  - softmax denominators (PSUM row 64) are copied to SBUF row 64 and
    broadcast to all 128 partitions by a selector matmul (lhsT = e64, a
    column that is 1 only on partition 64), then fast-reciprocal'd and
    multiplied into OT. No DMA anywhere in the normalization chain --
    DMA completion semaphores are batch-delivered and would stall it.
  - yp partials are stored bf16 (halves store DMA); host sums in fp32.
  - scheduling: chains are software-pipelined (the next chain's two
    score matmuls are emitted before the previous chain's stash/norm
    trailer) and phase-1/projection units are injected between
    attention units as PE filler -- V(tb) into block tb's own first
    slots, QK(tb+1) and proj(tb-1) spread across the rest -- because
    the exp chain on ACT (~940ns/chunk vs ~850ns PE/chunk) is the pacer.
  - startup DMAs are half-tensor granularity on two queues (Sync: W,
    Scalar: x) -- completion sems arrive in coarse batches, so chunked
    DMAs stall consumers; the final projection's copies split ACT/DVE
    and its stores split Sync/GpSimd so the tail drains at 2x.

The causal mask is handled by skipping fully-masked k-chunks and the
indicator multiply on diagonal-straddling chunks. If the runtime mask is
not the lower-tri causal mask, a general fallback multiplies by the
actual mask (DMA'd transposed) instead; an all-ones mask drops masking
entirely.
"""

import numpy as np

B, T, C, H = 2, 2048, 1024, 16
D = C // H            # 64 head dim
NCORES = 8
NBG = 2               # batch shards
NHG = 4               # head-group shards
HL = H // NHG         # 4 heads per core
DL = HL * D           # 256 local feature dims
NDQ = DL // 128       # 2 partition chunks of local dims
NTB = T // 512        # 4 t-chunks of 512
NKC = T // 128        # 16 key chunks of 128
NQC = T // 512        # 4 query chunks of 512
NTT = T // 128        # 16 t-tiles of 128 (proj / V)

_CACHE = {}


def _build(mode, has_bias, debug_dump=False):
    """Build + compile the per-core Bass program. mode: causal|full|general."""
    import concourse.bass as bass
    import concourse.bacc as bacc
    import concourse.tile as tile
    import concourse.mybir as mybir

    f32 = mybir.dt.float32
    bf16 = mybir.dt.bfloat16
    Exp = mybir.ActivationFunctionType.Exp
    mult = mybir.AluOpType.mult
    add = mybir.AluOpType.add

    nc = bacc.Bacc(
        "TRN2", target_bir_lowering=False, debug=False, num_devices=NCORES
    )

    xT = nc.dram_tensor("xT", [C, T], bf16, kind="ExternalInput").ap()
    Wl = nc.dram_tensor("Wl", [C, 3 * DL], bf16, kind="ExternalInput").ap()
    bqk = nc.dram_tensor("bqk", [128, 2 * NDQ], f32, kind="ExternalInput").ap()
    bv = nc.dram_tensor("bv", [1, DL], f32, kind="ExternalInput").ap()
    Wp = nc.dram_tensor("Wp", [DL, C], bf16, kind="ExternalInput").ap()
    e64d = nc.dram_tensor("e64d", [D + 1, D], bf16, kind="ExternalInput").ap()
    maskT = None
    if mode == "general":
        maskT = nc.dram_tensor("maskT", [T, T], bf16, kind="ExternalInput").ap()
    yp = nc.dram_tensor("yp", [T, C], bf16, kind="ExternalOutput").ap()
    dbg = {}
    if debug_dump:
        for nm, shp, dt in [
            ("qt_d", [128, NDQ, T], bf16), ("kt_d", [128, NDQ, T], bf16),
            ("v1_d", [128, NKC, HL, D + 1], bf16), ("ot_d", [128, NDQ, T], bf16),
        ]:
            dbg[nm] = nc.dram_tensor(nm, shp, dt, kind="ExternalOutput").ap()

    with tile.TileContext(nc) as tc:
        with (
            tc.tile_pool(name="singles", bufs=1) as singles,
            tc.tile_pool(name="xin", bufs=2) as xin,
            tc.tile_pool(name="ptiles", bufs=6) as ptiles,
            tc.tile_pool(name="small", bufs=4) as small,
            tc.tile_pool(name="outp", bufs=3) as outp,
            tc.tile_pool(name="psum", bufs=7, space="PSUM") as psum,
        ):
            def ps512(name):
                return psum.tile(
                    [128, 512], f32, name="ps512", tag="ps512", bufs=4
                )

            # ---- resident inputs ----
            # W and x loads split per kc-chunk so the first matmuls can
            # start as soon as their chunk lands; W on the sync DMA queue,
            # x on the scalar queue so they stream in parallel.
            W_sb = singles.tile([128, 8, 3 * DL], bf16)
            Wl_r = Wl.rearrange("(kc p) n -> p kc n", p=128)
            xr = xT.rearrange("(kc p) t -> p kc t", p=128)
            x_tiles = [
                xin.tile([128, 8, 512], bf16, tag="x_sb", name="x_sb", bufs=4)
                for _ in range(NTB)
            ]
            # DMA completions are delivered in coarse batches, so
            # fine-grained chunk DMAs stall the consumer pipeline on
            # semaphore delivery; use half-tensor transfers instead.
            for h in range(2):
                nc.sync.dma_start(
                    out=W_sb[:, 4 * h : 4 * h + 4, 0 : 2 * DL],
                    in_=Wl_r[:, 4 * h : 4 * h + 4, 0 : 2 * DL],
                )
                nc.scalar.dma_start(
                    out=x_tiles[0][:, 4 * h : 4 * h + 4, :],
                    in_=xr[:, 4 * h : 4 * h + 4, 0:512],
                )
            nc.sync.dma_start(
                out=W_sb[:, :, 2 * DL : 3 * DL],
                in_=Wl_r[:, :, 2 * DL : 3 * DL],
            )
            for h in range(2):
                nc.scalar.dma_start(
                    out=x_tiles[1][:, 4 * h : 4 * h + 4, :],
                    in_=xr[:, 4 * h : 4 * h + 4, 512:1024],
                )
            for tb in (2, 3):
                nc.sync.dma_start(
                    out=x_tiles[tb],
                    in_=xr[:, :, tb * 512 : (tb + 1) * 512],
                )
            bqk_sb = singles.tile([128, 2 * NDQ], f32)
            bv_row = singles.tile([1, DL], f32)
            bv_sb = singles.tile([128, DL], f32)
            if has_bias:
                nc.sync.dma_start(out=bqk_sb, in_=bqk)
                nc.sync.dma_start(out=bv_row, in_=bv)
                nc.gpsimd.partition_broadcast(bv_sb, bv_row)

            # partition-64 selector column for the denominator
            # broadcast: e64[p, m] = 1 iff p == 64, so
            # (e64^T @ rhs)[m, :] = rhs[64, :] for all m.
            e64_sb = singles.tile([D + 1, D], bf16)
            nc.scalar.dma_start(out=e64_sb, in_=e64d)
            # denominator staging tiles: row 64 gets the live values,
            # rows 0..63 must stay zero (they feed the selector matmul)
            den_tiles = [
                small.tile([D + 1, 2, 512], bf16, tag=f"den{i}", name=f"den{i}")
                for i in range(2)
            ]
            for dt_ in den_tiles:
                nc.gpsimd.memset(dt_, 0.0)

            ind = None
            if mode == "causal":
                ind = singles.tile([128, 4, 512], bf16)
                for j in range(4):
                    nc.vector.memset(ind[:, j, :], 1.0)
                    # keep (=1.0) iff f - p - 128*j >= 0, else 0.0
                    nc.gpsimd.affine_select(
                        out=ind[:, j, :],
                        in_=ind[:, j, :],
                        compare_op=mybir.AluOpType.is_ge,
                        fill=0.0,
                        base=-128 * j,
                        pattern=[[1, 512]],
                        channel_multiplier=-1,
                    )

            # ---- resident intermediates ----
            QT = singles.tile([128, NDQ, T], bf16)   # [dim%128, dimchunk, t]
            KT = singles.tile([128, NDQ, T], bf16)
            V1 = singles.tile([128, NKC, HL, D + 1], bf16)  # [t%128, kc, h, d+1]
            nc.vector.memset(V1[:, :, :, D : D + 1], 1.0)
            OT = singles.tile([128, NDQ, T], bf16)
            Wp_sb = singles.tile([128, NDQ, C], bf16)

            # ---- phase 1: QKV projections (as interleavable units) ----
            def p1_units(tb, x_sb):
                """QK and V units for one 512-wide t-chunk of the QKV
                projection, returned separately: V(tb) is only needed by
                block tb's own diagonal chunks, so it can inject there.
                PSUM->SBUF copies stay on DVE."""
                ceng = nc.vector.tensor_copy
                units = []
                for s in range(2):  # 0=Q, 1=K
                    for dq in range(NDQ):
                        shared = {}

                        def qk_a(tb=tb, s=s, dq=dq, x_sb=x_sb, shared=shared):
                            ps = ps512("qk")
                            shared["ps"] = ps
                            col = s * DL + dq * 128
                            for kc in range(4):
                                nc.tensor.matmul(
                                    ps,
                                    lhsT=W_sb[:, kc, col : col + 128],
                                    rhs=x_sb[:, kc, :],
                                    start=(kc == 0),
                                    stop=False,
                                )

                        def qk_b(tb=tb, s=s, dq=dq, x_sb=x_sb, shared=shared):
                            ps = shared.pop("ps")
                            col = s * DL + dq * 128
                            for kc in range(4, 8):
                                nc.tensor.matmul(
                                    ps,
                                    lhsT=W_sb[:, kc, col : col + 128],
                                    rhs=x_sb[:, kc, :],
                                    start=False,
                                    stop=(kc == 7),
                                )
                            dst = (QT if s == 0 else KT)[
                                :, dq, tb * 512 : (tb + 1) * 512
                            ]
                            if has_bias:
                                nc.vector.tensor_scalar_add(
                                    dst, ps,
                                    bqk_sb[:, s * NDQ + dq : s * NDQ + dq + 1],
                                )
                            else:
                                ceng(dst, ps)

                        units.append(qk_a)
                        units.append(qk_b)
                vunits = []
                for t4 in range(4):
                    def v_u(tb=tb, t4=t4, x_sb=x_sb):
                        tt = tb * 4 + t4
                        ps = ps512("v")
                        for kc in range(8):
                            nc.tensor.matmul(
                                ps[:, :DL],
                                lhsT=x_sb[:, kc, t4 * 128 : (t4 + 1) * 128],
                                rhs=W_sb[:, kc, 2 * DL : 3 * DL],
                                start=(kc == 0),
                                stop=(kc == 7),
                            )
                        if has_bias:
                            nc.vector.tensor_tensor(
                                out=V1[:, tt, :, 0:D],
                                in0=ps[:, :DL].rearrange(
                                    "p (h d) -> p h d", d=D
                                ),
                                in1=bv_sb.rearrange("p (h d) -> p h d", d=D),
                                op=add,
                            )
                        else:
                            ceng(
                                V1[:, tt, :, 0:D],
                                ps[:, :DL].rearrange("p (h d) -> p h d", d=D),
                            )
                    vunits.append(v_u)
                return units, vunits

            # ---- phase 2/3 units ----
            def proj_units(qc):
                # proj(qc) is injected during attention(qc+1); its
                # PSUM->SBUF copies run on DVE except the final proj,
                # which runs after the exp chain ends: there the copies
                # split ACT/DVE and the stores split Sync/GpSimd so the
                # drain pipelines at 2x.
                last = qc == NQC - 1
                units = []
                for t4 in range(4):
                    def u(qc=qc, t4=t4, last=last):
                        tt = qc * 4 + t4
                        y_sb = outp.tile([128, C], bf16, name="y_sb")
                        for n in range(2):
                            pp = ps512("proj")
                            for dq in range(NDQ):
                                nc.tensor.matmul(
                                    pp,
                                    lhsT=OT[:, dq, tt * 128 : (tt + 1) * 128],
                                    rhs=Wp_sb[:, dq, n * 512 : (n + 1) * 512],
                                    start=(dq == 0),
                                    stop=(dq == NDQ - 1),
                                )
                            if last:
                                ceng = (
                                    nc.scalar.copy
                                    if n == 0
                                    else nc.vector.tensor_copy
                                )
                                deng = nc.sync if n == 0 else nc.gpsimd
                            else:
                                ceng = nc.vector.tensor_copy
                                deng = nc.sync
                            ceng(y_sb[:, n * 512 : (n + 1) * 512], pp)
                            deng.dma_start(
                                out=yp[
                                    tt * 128 : (tt + 1) * 128,
                                    n * 512 : (n + 1) * 512,
                                ],
                                in_=y_sb[:, n * 512 : (n + 1) * 512],
                            )
                    units.append(u)
                return units

            def attn_units(qc, hp, nkc, m_sb, tail=False):
                """One head-pair's attention over all k-chunks, softmax
                denominators via the ones-column of V1. Diagonal chunks
                (kc - 4*qc = j >= 0 in causal mode) only touch score
                columns >= 128*j."""
                state = {}

                def emit_mm1(kc):
                    j = kc - 4 * qc if mode == "causal" else -1
                    lo = 128 * j if j > 0 else 0
                    stp = psum.tile(
                        [128, 2, 512], f32, name="ps1024", tag="ps1024", bufs=2
                    )
                    for hh in range(2):
                        off = 64 * hh
                        nc.tensor.matmul(
                            stp[:, hh, lo:],
                            lhsT=KT[off : off + 64, hp, kc * 128 : (kc + 1) * 128],
                            rhs=QT[
                                off : off + 64,
                                hp,
                                qc * 512 + lo : (qc + 1) * 512,
                            ],
                            start=True,
                            stop=True,
                        )
                    state.setdefault("st", {})[kc] = stp

                def prologue():
                    state["ops"] = [ps512("o"), ps512("o")]
                    state["emitted"] = min(2, nkc)  # lookahead 1
                    for j in range(state["emitted"]):
                        emit_mm1(j)

                def consume(kc):
                    ops = state["ops"]
                    stp = state["st"].pop(kc)
                    p2 = ptiles.tile([128, 2, 512], bf16, tag="p")
                    j = kc - 4 * qc if mode == "causal" else -1
                    lo = 128 * j if j > 0 else 0
                    if mode == "causal" and j >= 0:
                        # exp only the reachable columns; multiply the
                        # 128-wide diagonal-straddling block by the 0/1
                        # indicator (both heads at once). Columns beyond
                        # the straddle block are fully unmasked.
                        nc.scalar.activation(
                            p2[:, :, lo:], stp[:, :, lo:], Exp
                        )
                        base = ind[:, j, lo : lo + 128]
                        ind2 = bass.AP(
                            tensor=base.tensor,
                            offset=base.offset,
                            ap=[base.ap[0], [0, 2], base.ap[1]],
                        )
                        nc.vector.tensor_tensor(
                            out=p2[:, :, lo : lo + 128],
                            in0=p2[:, :, lo : lo + 128],
                            in1=ind2,
                            op=mult,
                        )
                    else:
                        nc.scalar.activation(p2, stp, Exp)
                        if mode == "general":
                            base = m_sb[:, kc, :]
                            msk2 = bass.AP(
                                tensor=base.tensor,
                                offset=base.offset,
                                ap=[base.ap[0], [0, 2], base.ap[1]],
                            )
                            nc.vector.tensor_tensor(
                                out=p2, in0=p2, in1=msk2, op=mult
                            )
                    for hh in range(2):
                        h = hp * 2 + hh
                        nc.tensor.matmul(
                            ops[hh][: D + 1, lo:],
                            lhsT=V1[:, kc, h, :],
                            rhs=p2[:, hh, lo:],
                            start=(kc == 0),
                            stop=(kc == nkc - 1),
                        )
                    if state["emitted"] < nkc:
                        emit_mm1(state["emitted"])
                        state["emitted"] += 1

                def stash():
                    # denominators first (they head the normalization
                    # chain), then the unnormalized output rows. The
                    # denominator row stays on partition 64; the selector
                    # matmul in norm_mm broadcasts it without any DMA.
                    # On the tail chain the copies split ACT/DVE so both
                    # denominators land immediately after the last exp.
                    ops = state["ops"]
                    den = den_tiles[(qc * NDQ + hp) % 2]
                    den_copy = [nc.vector.tensor_copy] * 2
                    ot_copy = [nc.vector.tensor_copy] * 2
                    if tail:
                        den_copy[0] = nc.scalar.copy
                        ot_copy = [nc.scalar.copy, nc.vector.tensor_copy]
                    for hh in range(2):
                        den_copy[hh](
                            den[64:65, hh, :], ops[hh][D : D + 1, :]
                        )
                    for hh in range(2):
                        off = 64 * hh
                        ot_copy[hh](
                            OT[off : off + 64, hp, qc * 512 : (qc + 1) * 512],
                            ops[hh][0:D, :],
                        )

                units = [(prologue, False)]
                for kc in range(nkc):
                    units.append((lambda kc=kc: consume(kc), True))
                units.append((stash, False))
                return units

            def norm_mm(qc, hp):
                # two selector matmuls broadcast the raw denominators to
                # all 128 partitions (64 per head) in PSUM, then one DVE
                # divide normalizes OT. No DMA, no reciprocal chain.
                den = den_tiles[(qc * NDQ + hp) % 2]
                rb_ps = ps512("rb")
                for hh in range(2):
                    nc.tensor.matmul(
                        rb_ps[64 * hh : 64 * hh + 64, :],
                        lhsT=e64_sb,
                        rhs=den[:, hh, :],
                        start=True,
                        stop=True,
                    )
                rb_sb = small.tile([128, 512], f32, tag="rb_sb", bufs=2)
                nc.vector.reciprocal_approx_fast(out=rb_sb, in_=rb_ps)
                nc.vector.tensor_tensor(
                    out=OT[:, hp, qc * 512 : (qc + 1) * 512],
                    in0=OT[:, hp, qc * 512 : (qc + 1) * 512],
                    in1=rb_sb,
                    op=mult,
                )

            pending_mm = []
            carry = []

            # ---- schedule: staircase interleave ----
            # attn(qc) needs phase-1 chunks tb <= qc only, so phase-1(tb+1)
            # and proj(qc-1) units are injected between attention units to
            # keep the PE FIFO fed while ACT paces the exp chain.
            qk0, v0 = p1_units(0, x_tiles[0])
            for u in qk0 + v0:
                u()
            p1_qk = {0: []}
            p1_v = {0: []}
            for tb in range(1, NTB):
                p1_qk[tb], p1_v[tb] = p1_units(tb, x_tiles[tb])
            if mode != "causal":
                # every chain consumes every k-chunk: all QKV projections
                # must be emitted before the attention loop
                for tb in range(1, NTB):
                    for u in p1_qk[tb] + p1_v[tb]:
                        u()
                    p1_qk[tb], p1_v[tb] = [], []
            nc.gpsimd.dma_start(
                out=Wp_sb, in_=Wp.rearrange("(dq p) n -> p dq n", p=128)
            )
            for qc in range(NQC):
                nkc = 4 * qc + 4 if mode == "causal" else NKC
                m_sb = None
                if mode == "general":
                    m_sb = xin.tile([128, NKC, 512], bf16, tag="mask", bufs=1)
                    nc.sync.dma_start(
                        out=m_sb,
                        in_=maskT.rearrange("(kc p) q -> p kc q", p=128)[
                            :, :, qc * 512 : (qc + 1) * 512
                        ],
                    )
                # V(qc) injects into this block's first consume slots
                # (its diagonal chunks need it); QK(qc+1) and the
                # previous block's projection spread over the rest.
                inj_early = p1_v.get(qc, [])
                inj_main = p1_qk.get(qc + 1, []) + (
                    proj_units(qc - 1) if qc >= 1 else []
                )
                main = []
                for hp in range(NDQ):
                    tail = qc == NQC - 1 and hp == NDQ - 1
                    units = attn_units(qc, hp, nkc, m_sb, tail=tail)

                    def pre_u(qc=qc, hp=hp):
                        pending_mm.append((qc, hp))

                    units.append((pre_u, False))
                    # software-pipeline across chains: this chain's
                    # prologue (2 mm1s) goes out BEFORE the previous
                    # chain's stash/norm trailer, so the PE always has
                    # score matmuls in flight while the first exp of the
                    # new chain is pending on ACT.
                    main.append(units[0])
                    main += carry
                    carry = units[-2:]
                    body = units[1:-2]
                    flush_at = min(2, len(body) - 1)
                    for i, u in enumerate(body):
                        main.append(u)
                        if i == flush_at:
                            def flush_u():
                                while pending_mm:
                                    q_, h_ = pending_mm.pop(0)
                                    norm_mm(q_, h_)
                            main.append((flush_u, False))
                slots = [i for i, (u, ok) in enumerate(main) if ok]
                place = {}
                for k, u in enumerate(inj_early):
                    place.setdefault(
                        slots[min(k, len(slots) - 1)], []
                    ).append(u)
                mslots = slots[4:] if len(slots) > 8 else slots
                for k, u in enumerate(inj_main):
                    idx = mslots[
                        min(
                            int((k + 0.5) * len(mslots) / max(len(inj_main), 1)),
                            len(mslots) - 1,
                        )
                    ]
                    place.setdefault(idx, []).append(u)
                for i, (u, ok) in enumerate(main):
                    u()
                    for inj in place.get(i, ()):
                        inj()
            for u, _ in carry:
                u()
            while pending_mm:
                q_, h_ = pending_mm.pop(0)
                norm_mm(q_, h_)
            for u in proj_units(NQC - 1):
                u()

            if debug_dump:
                nc.sync.dma_start(out=dbg["ot_d"], in_=OT)

    nc.compile()
    return nc


def _host_prep(x, prefix_causal_mask, W_attn, b_attn, W_proj):
    """Split full inputs into 8 per-core input maps; detect mask mode."""
    scale = 1.0 / np.sqrt(np.float32(D))
    mask = np.asarray(prefix_causal_mask)
    if mask.all():
        mode = "full"
    else:
        tri = np.tril(np.ones((T, T), dtype=bool))
        if all(np.array_equal(mask[b], tri) for b in range(B)):
            mode = "causal"
        else:
            mode = "general"

    import ml_dtypes

    bf16 = ml_dtypes.bfloat16
    x = np.asarray(x, dtype=np.float32)
    W_attn = np.asarray(W_attn, dtype=np.float32)
    b_attn = np.asarray(b_attn, dtype=np.float32)
    W_proj = np.asarray(W_proj, dtype=np.float32)
    has_bias = bool(np.any(b_attn))

    in_maps = []
    for core in range(NCORES):
        b = core // NHG
        hg = core % NHG
        lo = hg * DL
        hi = lo + DL
        xT = np.ascontiguousarray(x[b].T)  # [C, T]
        Wq = W_attn[:, lo:hi] * scale
        Wk = W_attn[:, C + lo : C + hi]
        Wv = W_attn[:, 2 * C + lo : 2 * C + hi]
        Wl = np.ascontiguousarray(np.concatenate([Wq, Wk, Wv], axis=1))
        bq = b_attn[lo:hi] * scale
        bk = b_attn[C + lo : C + hi]
        # bias per partition for Q,K chunks: cols = [q0, q1, k0, k1]
        bqk = np.stack(
            [bq[0:128], bq[128:256], bk[0:128], bk[128:256]], axis=1
        ).astype(np.float32)
        bv = np.ascontiguousarray(
            b_attn[2 * C + lo : 2 * C + hi][None, :]
        ).astype(np.float32)
        Wp = np.ascontiguousarray(W_proj[lo:hi, :])
        e64 = np.zeros((D + 1, D), dtype=np.float32)
        e64[D, :] = 1.0
        im = {
            "xT": xT.astype(bf16),
            "Wl": Wl.astype(bf16),
            "bqk": np.ascontiguousarray(bqk),
            "bv": bv,
            "Wp": Wp.astype(bf16),
            "e64d": e64.astype(bf16),
        }
        if mode == "general":
            im["maskT"] = np.ascontiguousarray(mask[b].T).astype(bf16)
        in_maps.append(im)
    return mode, has_bias, in_maps


def _get_program(mode, has_bias):
    key = (mode, has_bias)
    if key not in _CACHE:
        _CACHE[key] = _build(mode, has_bias)
    return _CACHE[key]


def _run(inputs, trace=False):
    """Returns (full_output [B,T,C], BassKernelResults)."""
    from concourse import bass_utils

    mode, has_bias, in_maps = _host_prep(
        inputs["x"],
        inputs["prefix_causal_mask"],
        inputs["W_attn"],
        inputs["b_attn"],
        inputs["W_proj"],
    )
    nc = _get_program(mode, has_bias)
    res = bass_utils.run_bass_kernel_spmd(
        nc, in_maps, core_ids=list(range(NCORES)), trace=trace
    )
    b_proj = np.asarray(inputs["b_proj"], dtype=np.float32)
    y = np.zeros((B, T, C), dtype=np.float32)
    for core in range(NCORES):
        y[core // NHG] += np.asarray(res.results[core]["yp"], dtype=np.float32)
    y += b_proj[None, None, :]
    return y, res


def kernel(**inputs):
    y, _ = _run(inputs, trace=False)
    return y
